# revision 1
# baseline (speedup 1.0000x reference)
"""Trainium2 Bass kernel for LogitBiasedSelfAttention1D.

Sharding: 8 cores = (batch b in 0..3) x (query half qh in 0..1).
Each core computes full attention (all 8 heads, all 2048 keys) for its
1024 queries of its batch. No collectives.

Math decomposition (exactly equivalent to the reference up to fp):
  - conv1d key bias folded into V:  softmax(S + bias) @ V
      = (exp(S) @ (c * V)) / (exp(S) @ c),   c = exp(bias)
  - SCALE folded into w_q on host.
  - b_out + residual x_seq folded into one host-prepared addend.
  - LayerNorm gamma/beta folded into the final transpose drain.
All matmuls in bf16; accumulation and softmax denominator in fp32.
"""

import sys

for _p in ("/opt/trn_rl_repo", "/root/.axon_site/_ro/trn_rl_repo"):
    if _p not in sys.path:
        sys.path.insert(0, _p)

import numpy as np
import ml_dtypes

from concourse import bass, mybir
from concourse.tile import TileContext
from concourse.bass_utils import run_bass_kernel_spmd

B, C, T = 4, 512, 2048
H, D = 8, 64
SCALE = D ** -0.5
EPS = 1e-5
TQ = T // 2            # queries per core
KC = T // 128          # 16 key chunks
PAIRS = H // 2         # 4 head pairs
F32 = mybir.dt.float32
BF16 = mybir.dt.bfloat16
bf16 = ml_dtypes.bfloat16

Exp = mybir.ActivationFunctionType.Exp
Sqrt = mybir.ActivationFunctionType.Sqrt
MULT = mybir.AluOpType.mult
ADD = mybir.AluOpType.add

_CACHE = {}


def _build_nc():
    nc = bass.Bass()
    xct = nc.declare_dram_parameter("xct", [C, T], BF16, False)       # x[b], (C,T)
    xq = nc.declare_dram_parameter("xq", [C, TQ], BF16, False)        # query slice of x[b]
    xseq = nc.declare_dram_parameter("xseq", [TQ, C], F32, False)     # x[b].T slice + b_out
    wq = nc.declare_dram_parameter("wq", [C, C], BF16, False)         # (c_in, c_out), * SCALE
    wk = nc.declare_dram_parameter("wk", [C, C], BF16, False)
    wv = nc.declare_dram_parameter("wv", [C, C], BF16, False)
    wo = nc.declare_dram_parameter("wo", [C, C], BF16, False)
    cful = nc.declare_dram_parameter("cful", [128, KC], F32, False)   # c chunks (key-major)
    c8 = nc.declare_dram_parameter("c8", [128, KC * H], BF16, False)  # c replicated per head
    gmm = nc.declare_dram_parameter("gmm", [128, 4], F32, False)
    bet = nc.declare_dram_parameter("bet", [128, 4], F32, False)
    iden = nc.declare_dram_parameter("iden", [128, 128], BF16, False)
    outp = nc.declare_dram_parameter("out", [C, TQ], F32, True)

    with TileContext(nc) as tc:
        with (
            tc.sbuf_pool(name="cst", bufs=1) as cst,
            tc.sbuf_pool(name="pex", bufs=2) as pex,
            tc.sbuf_pool(name="sml", bufs=2) as sml,
            tc.psum_pool(name="ps", bufs=1) as ps,
        ):
            # ---- constants / persistent tiles ----
            XCT = [cst.tile_from(xct[i * 128:(i + 1) * 128, :], name=f"XCT{i}")
                   for i in range(4)]
            XQ = [cst.tile_from(xq[i * 128:(i + 1) * 128, :], name=f"XQ{i}")
                  for i in range(4)]
            XS = [cst.tile_from(xseq[t * 128:(t + 1) * 128, :], name=f"XS{t}")
                  for t in range(8)]
            WQ = [cst.tile_from(wq[i * 128:(i + 1) * 128, :], name=f"WQ{i}")
                  for i in range(4)]
            WK = [cst.tile_from(wk[i * 128:(i + 1) * 128, :], name=f"WK{i}")
                  for i in range(4)]
            WV = [cst.tile_from(wv[i * 128:(i + 1) * 128, :], name=f"WV{i}")
                  for i in range(4)]
            WO = [cst.tile_from(wo[i * 128:(i + 1) * 128, :], name=f"WO{i}")
                  for i in range(4)]
            CF = cst.tile_from(cful[:, :], name="CF")
            C8 = cst.tile_from(c8[:, :], name="C8")
            GM = cst.tile_from(gmm[:, :], name="GM")
            BT = cst.tile_from(bet[:, :], name="BT")
            ID = cst.tile_from(iden[:, :], name="ID")

            epsT = cst.tile([128, 1], F32, name="epsT")
            nc.vector.memset(epsT[:, :], EPS)
            KT = [cst.tile([128, T], BF16, name=f"KT{m}") for m in range(4)]
            QT = [cst.tile([128, TQ], BF16, name=f"QT{m}") for m in range(4)]
            VB = [cst.tile([128, H * 65], BF16, name=f"VB{k}") for k in range(KC)]
            OT = [cst.tile([128, TQ], BF16, name=f"OTp{p}") for p in range(PAIRS)]
            OUTS = [cst.tile([128, TQ], F32, name=f"OUTS{c_}") for c_ in range(4)]

            # ---- K^T projection: KT[m] = (wk.T x)[m-chunk]  (c_out, tokens) ----
            for m in range(4):
                for h2 in range(2):
                    kps = ps.tile([128, 1024], F32, tag=("SA", "SB")[(m * 2 + h2) % 2],
                                  name=f"kps{m}_{h2}")
                    for n in range(2):
                        for ci in range(4):
                            nc.tensor.matmul(
                                kps[:, n * 512:(n + 1) * 512],
                                lhsT=WK[ci][:, m * 128:(m + 1) * 128],
                                rhs=XCT[ci][:, h2 * 1024 + n * 512: h2 * 1024 + (n + 1) * 512],
                                start=(ci == 0), stop=(ci == 3))
                    nc.vector.tensor_copy(KT[m][:, h2 * 1024:(h2 + 1) * 1024], kps[:, :])

            # ---- Q^T projection (queries only) ----
            for m in range(4):
                qps = ps.tile([128, 1024], F32, tag=("SA", "SB")[m % 2], name=f"qps{m}")
                for n in range(2):
                    for ci in range(4):
                        nc.tensor.matmul(
                            qps[:, n * 512:(n + 1) * 512],
                            lhsT=WQ[ci][:, m * 128:(m + 1) * 128],
                            rhs=XQ[ci][:, n * 512:(n + 1) * 512],
                            start=(ci == 0), stop=(ci == 3))
                nc.vector.tensor_copy(QT[m][:, :], qps[:, :])

            # ---- V projection, c-scaled, (keys, heads*65) blocks ----
            for k in range(KC):
                vps = ps.tile([128, 512], F32, tag=("OA1", "OB1")[k % 2], name=f"vps{k}")
                for ci in range(4):
                    nc.tensor.matmul(
                        vps[:, :],
                        lhsT=XCT[ci][:, k * 128:(k + 1) * 128],
                        rhs=WV[ci][:, :],
                        start=(ci == 0), stop=(ci == 3))
                nc.vector.tensor_scalar(
                    out=VB[k].rearrange("p (h e) -> p h e", e=65)[:, :, 0:64],
                    in0=vps.rearrange("p (h e) -> p h e", e=64),
                    scalar1=CF[:, k:k + 1], scalar2=None, op0=MULT)
                nc.vector.tensor_copy(
                    VB[k].rearrange("p (h e) -> p h e", e=65)[:, :, 64:65],
                    C8[:, k * H:(k + 1) * H].rearrange("p (h e) -> p h e", e=1))

            # ---- attention per head pair ----
            for p in range(PAIRS):
                oacc = {}
                for hi, tags in ((0, ("OA1", "OA2")), (1, ("OB1", "OB2"))):
                    oacc[hi] = [ps.tile([128, 512], F32, tag=tags[bk],
                                        name=f"o{p}_{hi}_{bk}") for bk in range(2)]
                for k in range(KC):
                    for hi, stag, ptag in ((0, "SA", "pA"), (1, "SB", "pB")):
                        rows = slice(hi * 64, (hi + 1) * 64)
                        s_ps = ps.tile([128, 1024], F32, tag=stag, name=f"s{p}_{hi}_{k}")
                        for n in range(2):
                            nc.tensor.matmul(
                                s_ps[:, n * 512:(n + 1) * 512],
                                lhsT=KT[p][rows, k * 128:(k + 1) * 128],
                                rhs=QT[p][rows, n * 512:(n + 1) * 512],
                                start=True, stop=True)
                        pt = pex.tile([128, 1024], BF16, tag=ptag, name=f"pt{p}_{hi}_{k}")
                        nc.scalar.activation(pt[:, :], s_ps[:, :], Exp)
                        head = 2 * p + hi
                        for s in range(8):
                            bk, j = s // 4, s % 4
                            nc.tensor.matmul(
                                oacc[hi][bk][:, j * 65:(j + 1) * 65],
                                lhsT=pt[:, s * 128:(s + 1) * 128],
                                rhs=VB[k][:, head * 65:(head + 1) * 65],
                                start=(k == 0), stop=(k == KC - 1))

                # epilogue: normalize by denominator, pack [A|B] columns
                ONs = [sml.tile([128, 128], BF16, tag="on", name=f"on{p}_{s}", bufs=10)
                       for s in range(8)]
                for hi in (0, 1):
                    for bk in range(2):
                        o_ps = oacc[hi][bk]
                        ov = o_ps[:, 0:260].rearrange("p (s e) -> p s e", e=65)
                        rd4 = sml.tile([128, 4], F32, tag="rd", name=f"rd{p}_{hi}_{bk}",
                                       bufs=4)
                        nc.vector.reciprocal(
                            rd4.rearrange("p (s e) -> p s e", e=1), ov[:, :, 64:65])
                        for j in range(4):
                            s = bk * 4 + j
                            nc.vector.tensor_scalar_mul(
                                ONs[s][:, hi * 64:(hi + 1) * 64],
                                o_ps[:, j * 65:j * 65 + 64],
                                rd4[:, j:j + 1])
                # transpose packed (queries, pair-channels) -> (channels, queries)
                tps = [ps.tile([128, 512], BF16, tag=("OA1", "OA2")[b_], name=f"tp{p}_{b_}")
                       for b_ in range(2)]
                for s in range(8):
                    nc.tensor.transpose(
                        tps[s // 4][:, (s % 4) * 128:(s % 4 + 1) * 128],
                        ONs[s][:, :], ID[:, :])
                for b_ in range(2):
                    nc.vector.tensor_copy(OT[p][:, b_ * 512:(b_ + 1) * 512], tps[b_][:, :])

            # ---- out_proj + residual + LN + final transpose ----
            for t in range(8):
                ops_ = ps.tile([128, 512], F32, tag=("OB1", "OB2")[t % 2], name=f"op{t}")
                for p in range(PAIRS):
                    nc.tensor.matmul(
                        ops_[:, :],
                        lhsT=OT[p][:, t * 128:(t + 1) * 128],
                        rhs=WO[p][:, :],
                        start=(p == 0), stop=(p == PAIRS - 1))
                # residual (+b_out already folded in xseq)
                nc.vector.tensor_tensor(ops_[:, :], ops_[:, :], XS[t][:, :], op=ADD)
                bnst = sml.tile([128, 6], F32, tag="bnst", name=f"bnst{t}", bufs=3)
                bnag = sml.tile([128, 2], F32, tag="bnag", name=f"bnag{t}", bufs=3)
                nc.vector.bn_stats(bnst[:, :], ops_[:, :])
                nc.vector.bn_aggr(bnag[:, :], bnst[:, :])
                std = sml.tile([128, 1], F32, tag="std", name=f"std{t}", bufs=3)
                nc.scalar.activation(std[:, :], bnag[:, 1:2], Sqrt, bias=epsT[:, :])
                rstd = sml.tile([128, 1], F32, tag="rstd", name=f"rstd{t}", bufs=3)
                nc.vector.reciprocal(rstd[:, :], std[:, :])
                nmr = sml.tile([128, 1], F32, tag="nmr", name=f"nmr{t}", bufs=3)
                nc.vector.tensor_scalar(out=nmr[:, :], in0=bnag[:, 0:1],
                                        scalar1=rstd[:, :], scalar2=-1.0,
                                        op0=MULT, op1=MULT)
                hn = sml.tile([128, C], BF16, tag="hn", name=f"hn{t}", bufs=3)
                nc.vector.tensor_scalar(out=hn[:, :], in0=ops_[:, :],
                                        scalar1=rstd[:, :], scalar2=nmr[:, :],
                                        op0=MULT, op1=ADD)
                ftp = ps.tile([128, 512], BF16, tag=("OA1", "OA2")[t % 2], name=f"ftp{t}")
                for cc in range(4):
                    nc.tensor.transpose(
                        ftp[:, cc * 128:(cc + 1) * 128],
                        hn[:, cc * 128:(cc + 1) * 128], ID[:, :])
                for cc in range(4):
                    nc.vector.tensor_scalar(
                        out=OUTS[cc][:, t * 128:(t + 1) * 128],
                        in0=ftp[:, cc * 128:(cc + 1) * 128],
                        scalar1=GM[:, cc:cc + 1], scalar2=BT[:, cc:cc + 1],
                        op0=MULT, op1=ADD)

            for cc in range(4):
                nc.sync.dma_start(out=outp[cc * 128:(cc + 1) * 128, :], in_=OUTS[cc][:, :])

    _split_mm_waits(nc)
    return nc


def _split_mm_waits(nc):
    """Walrus MM structs carry only one sync wait; move extras to a NoOp."""
    f = nc.m.functions[0]
    for bb in f.blocks:
        il = bb.instructions
        out, changed = [], False
        for i in il:
            si = getattr(i, "sync_info", None)
            tn = type(i).__name__
            splittable = tn.startswith("Inst") and tn not in ("InstNoOp", "InstAllEngineBarrier")
            if (splittable and si is not None
                    and si.on_wait is not None and len(si.on_wait) > 1):
                waits = list(si.on_wait)
                for wi, w in enumerate(waits[:-1]):
                    out.append(mybir.InstNoOp(
                        name=f"{i.name}-wsplit{wi}", engine=i.engine,
                        sync_info=mybir.SyncInfo(on_wait=[w], on_update=[])))
                i.sync_info = mybir.SyncInfo(
                    on_wait=[waits[-1]], on_update=list(si.on_update))
                changed = True
            out.append(i)
        if changed:
            bb.instructions = out


def _prep_inputs(x, sqi, w_qkv, w_out, b_out, w_conv, b_conv, ln_gamma, ln_beta):
    x = np.asarray(x, np.float32)
    sqi = np.asarray(sqi, np.float32)
    w_qkv = np.asarray(w_qkv, np.float32)
    w_out = np.asarray(w_out, np.float32)
    b_out = np.asarray(b_out, np.float32)
    w_conv = np.asarray(w_conv, np.float32)
    b_conv = np.asarray(b_conv, np.float32)
    ln_gamma = np.asarray(ln_gamma, np.float32)
    ln_beta = np.asarray(ln_beta, np.float32)

    sp = np.pad(sqi, ((0, 0), (1, 1)))
    bias = (w_conv[0] * sp[:, :-2] + w_conv[1] * sp[:, 1:-1]
            + w_conv[2] * sp[:, 2:] + b_conv)                    # (B, T)
    c = np.exp(bias).astype(np.float32)

    wqT = (w_qkv[:C].T * SCALE).astype(bf16)
    wkT = w_qkv[C:2 * C].T.astype(bf16)
    wvT = w_qkv[2 * C:].T.astype(bf16)
    woT = w_out.T.astype(bf16)
    gm = ln_gamma.reshape(4, 128).T.copy().astype(np.float32)
    bt = ln_beta.reshape(4, 128).T.copy().astype(np.float32)
    iden = np.eye(128, dtype=bf16)

    in_maps = []
    for core in range(8):
        b, qh = divmod(core, 2)
        qs = slice(qh * TQ, (qh + 1) * TQ)
        cb = c[b]
        cful = cb.reshape(KC, 128).T.copy().astype(np.float32)
        c8 = np.repeat(cb.reshape(KC, 128).T, H, axis=1).copy().astype(bf16)
        in_maps.append({
            "xct": x[b].astype(bf16),
            "xq": x[b][:, qs].copy().astype(bf16),
            "xseq": (x[b].T[qs] + b_out).copy().astype(np.float32),
            "wq": wqT, "wk": wkT, "wv": wvT, "wo": woT,
            "cful": cful, "c8": c8, "gmm": gm, "bet": bt, "iden": iden,
        })
    return in_maps


def kernel(x, sqi, w_qkv, w_out, b_out, w_conv, b_conv, ln_gamma, ln_beta,
           _trace=False):
    if "nc" not in _CACHE:
        _CACHE["nc"] = _build_nc()
    nc = _CACHE["nc"]
    in_maps = _prep_inputs(x, sqi, w_qkv, w_out, b_out, w_conv, b_conv,
                           ln_gamma, ln_beta)
    res = run_bass_kernel_spmd(nc, in_maps, core_ids=list(range(8)), trace=_trace)
    _CACHE["last_result"] = res
    out = np.empty((B, C, T), np.float32)
    for core in range(8):
        b, qh = divmod(core, 2)
        out[b][:, qh * TQ:(qh + 1) * TQ] = res.results[core]["out"]
    return out



# revision 3
# speedup vs baseline: 1.1500x; 1.1500x over previous
"""Trainium2 Bass kernel for LogitBiasedSelfAttention1D.

Sharding: 8 cores = (batch b in 0..3) x (query half qh in 0..1).
Each core computes full attention (all 8 heads, all 2048 keys) for the
1024 queries of its batch half. No collectives.

Math decomposition (exactly equivalent to the reference up to fp):
  - conv1d key bias applied inside the softmax exp via the activation
    engine's per-partition bias operand: pt = exp(S + bias_key).
  - PV computed transposed (O^T = V^T P) so V is the stationary matmul
    operand: out psum rows 0..63 = head output (d, queries), row 64 =
    softmax denominator (V tile carries a ones column).
  - normalization: DVE reciprocal of the denominator row, stride-0
    broadcast DMA across partitions, DVE multiply into (c_in, queries)
    layout consumed directly by out_proj.
  - SCALE folded into w_q on host; b_out + residual folded into one
    host-prepared addend; LN gamma/beta folded into the final
    transpose drain.
All matmuls in bf16; accumulation and softmax denominator in fp32.
"""

import sys

for _p in ("/opt/trn_rl_repo", "/root/.axon_site/_ro/trn_rl_repo"):
    if _p not in sys.path:
        sys.path.insert(0, _p)

import numpy as np
import ml_dtypes

from concourse import bass, mybir
from concourse.tile import TileContext
from concourse.bass_utils import run_bass_kernel_spmd

B, C, T = 4, 512, 2048
H, D = 8, 64
SCALE = D ** -0.5
EPS = 1e-5
TQ = T // 2            # queries per core
KC = T // 128          # 16 key chunks
PAIRS = H // 2         # 4 head pairs
F32 = mybir.dt.float32
BF16 = mybir.dt.bfloat16
bf16 = ml_dtypes.bfloat16

Exp = mybir.ActivationFunctionType.Exp
Sqrt = mybir.ActivationFunctionType.Sqrt
MULT = mybir.AluOpType.mult
ADD = mybir.AluOpType.add

_CACHE = {}


def _build_nc():
    nc = bass.Bass()
    xct = nc.declare_dram_parameter("xct", [C, T], BF16, False)       # x[b], (C,T)
    xq = nc.declare_dram_parameter("xq", [C, TQ], BF16, False)        # query slice of x[b]
    xseq = nc.declare_dram_parameter("xseq", [TQ, C], F32, False)     # x[b].T slice + b_out
    wq = nc.declare_dram_parameter("wq", [C, C], BF16, False)         # (c_in, c_out), * SCALE
    wk = nc.declare_dram_parameter("wk", [C, C], BF16, False)
    wv = nc.declare_dram_parameter("wv", [C, C], BF16, False)
    wo = nc.declare_dram_parameter("wo", [C, C], BF16, False)
    cbp = nc.declare_dram_parameter("cb", [128, KC], F32, False)      # conv bias per key
    gmm = nc.declare_dram_parameter("gmm", [128, 4], F32, False)
    bet = nc.declare_dram_parameter("bet", [128, 4], F32, False)
    iden = nc.declare_dram_parameter("iden", [128, 128], BF16, False)
    outp = nc.declare_dram_parameter("out", [C, TQ], F32, True)

    with TileContext(nc) as tc:
        with (
            tc.sbuf_pool(name="cst", bufs=1) as cst,
            tc.sbuf_pool(name="pex", bufs=1) as pex,
            tc.sbuf_pool(name="sml", bufs=1) as sml,
            tc.psum_pool(name="ps", bufs=1) as ps,
        ):
            # ---- constants / persistent tiles ----
            XCT = [cst.tile_from(xct[i * 128:(i + 1) * 128, :], name=f"XCT{i}")
                   for i in range(4)]
            XQ = [cst.tile_from(xq[i * 128:(i + 1) * 128, :], name=f"XQ{i}")
                  for i in range(4)]
            WK = [cst.tile_from(wk[i * 128:(i + 1) * 128, :], name=f"WK{i}")
                  for i in range(4)]
            WQ = [cst.tile_from(wq[i * 128:(i + 1) * 128, :], name=f"WQ{i}")
                  for i in range(4)]
            WV = [cst.tile_from(wv[i * 128:(i + 1) * 128, :], name=f"WV{i}")
                  for i in range(4)]
            CB = cst.tile_from(cbp[:, :], name="CB")
            WO = [cst.tile_from(wo[i * 128:(i + 1) * 128, :], name=f"WO{i}")
                  for i in range(4)]
            XS = [cst.tile_from(xseq[t * 128:(t + 1) * 128, :], name=f"XS{t}")
                  for t in range(8)]
            GM = cst.tile_from(gmm[:, :], name="GM")
            BT = cst.tile_from(bet[:, :], name="BT")
            ID = cst.tile_from(iden[:, :], name="ID")

            epsT = cst.tile([128, 1], F32, name="epsT")
            nc.vector.memset(epsT[:, :], EPS)
            KT = [cst.tile([128, T], BF16, name=f"KT{m}") for m in range(4)]
            QT = [cst.tile([128, TQ], BF16, name=f"QT{m}") for m in range(4)]
            VB = [cst.tile([128, H * 65], BF16, name=f"VB{k}") for k in range(KC)]
            OT = [cst.tile([128, TQ], BF16, name=f"OTp{p}") for p in range(PAIRS)]
            OUTS = [cst.tile([128, TQ], F32, name=f"OUTS{c_}") for c_ in range(4)]

            # ones column per head (softmax denominator accumulator row)
            for k in range(KC):
                nc.vector.memset(
                    VB[k].rearrange("p (h e) -> p h e", e=65)[:, :, 64:65], 1.0)

            # ---- K^T projection: KT[m] = (wk.T x)[m-chunk]  (c_out, tokens) ----
            for m in range(4):
                for h2 in range(2):
                    kps = ps.tile([128, 1024], F32, tag="S", bufs=2,
                                  name=f"kps{m}_{h2}")
                    for n in range(2):
                        for ci in range(4):
                            nc.tensor.matmul(
                                kps[:, n * 512:(n + 1) * 512],
                                lhsT=WK[ci][:, m * 128:(m + 1) * 128],
                                rhs=XCT[ci][:, h2 * 1024 + n * 512: h2 * 1024 + (n + 1) * 512],
                                start=(ci == 0), stop=(ci == 3))
                    nc.vector.tensor_copy(KT[m][:, h2 * 1024:(h2 + 1) * 1024], kps[:, :])

            # ---- Q^T projection (queries only) ----
            for m in range(4):
                qps = ps.tile([128, 1024], F32, tag="S", bufs=2, name=f"qps{m}")
                for n in range(2):
                    for ci in range(4):
                        nc.tensor.matmul(
                            qps[:, n * 512:(n + 1) * 512],
                            lhsT=WQ[ci][:, m * 128:(m + 1) * 128],
                            rhs=XQ[ci][:, n * 512:(n + 1) * 512],
                            start=(ci == 0), stop=(ci == 3))
                nc.vector.tensor_copy(QT[m][:, :], qps[:, :])

            # ---- V projection, (keys, heads*65) blocks ----
            for k in range(KC):
                vps = ps.tile([128, 512], F32, tag="PO", bufs=2, name=f"vps{k}")
                for ci in range(4):
                    nc.tensor.matmul(
                        vps[:, :],
                        lhsT=XCT[ci][:, k * 128:(k + 1) * 128],
                        rhs=WV[ci][:, :],
                        start=(ci == 0), stop=(ci == 3))
                nc.vector.tensor_copy(
                    VB[k].rearrange("p (h e) -> p h e", e=65)[:, :, 0:64],
                    vps.rearrange("p (h e) -> p h e", e=64))

            # ---- attention: per (pair, head-in-pair), O^T accumulation ----
            for p in range(PAIRS):
                for hi in range(2):
                    head = 2 * p + hi
                    rows = slice(hi * 64, (hi + 1) * 64)
                    po = ps.tile([128, 1024], F32, tag="PO", bufs=2,
                                 name=f"po{p}_{hi}")
                    for k in range(KC):
                        s_ps = ps.tile([128, 1024], F32, tag="S", bufs=2,
                                       name=f"s{p}_{hi}_{k}")
                        for n in range(2):
                            nc.tensor.matmul(
                                s_ps[:, n * 512:(n + 1) * 512],
                                lhsT=KT[p][rows, k * 128:(k + 1) * 128],
                                rhs=QT[p][rows, n * 512:(n + 1) * 512],
                                start=True, stop=True)
                        pt = pex.tile([128, 1024], BF16, tag="pt", bufs=2,
                                      name=f"pt{p}_{hi}_{k}")
                        nc.scalar.activation(pt[:, :], s_ps[:, :], Exp,
                                             bias=CB[:, k:k + 1])
                        for n in range(2):
                            nc.tensor.matmul(
                                po[0:65, n * 512:(n + 1) * 512],
                                lhsT=VB[k][:, head * 65:(head + 1) * 65],
                                rhs=pt[:, n * 512:(n + 1) * 512],
                                start=(k == 0), stop=(k == KC - 1))
                    # normalize: rows 0..63 / row 64, into out_proj layout
                    rden = sml.tile([1, 1024], F32, tag="rden", bufs=2,
                                    name=f"rden{p}_{hi}")
                    nc.vector.reciprocal(rden[:, :], po[64:65, :])
                    rb = sml.tile([64, 1024], F32, tag="rb", bufs=2,
                                  name=f"rb{p}_{hi}")
                    # partition-broadcast via DMA: free-dim stride-0 repeat
                    # of the single source row (partition dim must have
                    # nonzero step, so the repeat lives in a free dim).
                    rden_bc = bass.AP(tensor=rden.tensor, offset=rden.offset,
                                      ap=[[1, 1], [0, 64], [1, 1024]])
                    nc.sync.dma_start(out=rb[:, :], in_=rden_bc)
                    nc.vector.tensor_tensor(
                        OT[p][hi * 64:(hi + 1) * 64, :],
                        po[0:64, :], rb[:, :], op=MULT)

            # ---- out_proj + residual + LN + final transpose ----
            for t in range(8):
                ops_ = ps.tile([128, 512], F32, tag="S", bufs=2, name=f"op{t}")
                for p in range(PAIRS):
                    nc.tensor.matmul(
                        ops_[:, :],
                        lhsT=OT[p][:, t * 128:(t + 1) * 128],
                        rhs=WO[p][:, :],
                        start=(p == 0), stop=(p == PAIRS - 1))
                # residual (+b_out already folded in xseq)
                nc.vector.tensor_tensor(ops_[:, :], ops_[:, :], XS[t][:, :], op=ADD)
                bnst = sml.tile([128, 6], F32, tag="bnst", name=f"bnst{t}", bufs=3)
                bnag = sml.tile([128, 2], F32, tag="bnag", name=f"bnag{t}", bufs=3)
                nc.vector.bn_stats(bnst[:, :], ops_[:, :])
                nc.vector.bn_aggr(bnag[:, :], bnst[:, :])
                std = sml.tile([128, 1], F32, tag="std", name=f"std{t}", bufs=3)
                nc.scalar.activation(std[:, :], bnag[:, 1:2], Sqrt, bias=epsT[:, :])
                rstd = sml.tile([128, 1], F32, tag="rstd", name=f"rstd{t}", bufs=3)
                nc.vector.reciprocal(rstd[:, :], std[:, :])
                nmr = sml.tile([128, 1], F32, tag="nmr", name=f"nmr{t}", bufs=3)
                nc.vector.tensor_scalar(out=nmr[:, :], in0=bnag[:, 0:1],
                                        scalar1=rstd[:, :], scalar2=-1.0,
                                        op0=MULT, op1=MULT)
                hn = sml.tile([128, C], BF16, tag="hn", name=f"hn{t}", bufs=3)
                nc.vector.tensor_scalar(out=hn[:, :], in0=ops_[:, :],
                                        scalar1=rstd[:, :], scalar2=nmr[:, :],
                                        op0=MULT, op1=ADD)
                ftp = ps.tile([128, 512], BF16, tag="PO", bufs=2, name=f"ftp{t}")
                for cc in range(4):
                    nc.tensor.transpose(
                        ftp[:, cc * 128:(cc + 1) * 128],
                        hn[:, cc * 128:(cc + 1) * 128], ID[:, :])
                for cc in range(4):
                    nc.vector.tensor_scalar(
                        out=OUTS[cc][:, t * 128:(t + 1) * 128],
                        in0=ftp[:, cc * 128:(cc + 1) * 128],
                        scalar1=GM[:, cc:cc + 1], scalar2=BT[:, cc:cc + 1],
                        op0=MULT, op1=ADD)

            for cc in range(4):
                nc.sync.dma_start(out=outp[cc * 128:(cc + 1) * 128, :], in_=OUTS[cc][:, :])

    _split_mm_waits(nc)
    return nc


def _split_mm_waits(nc):
    """Walrus MM structs carry only one sync wait; move extras to a NoOp."""
    f = nc.m.functions[0]
    for bb in f.blocks:
        il = bb.instructions
        out, changed = [], False
        for i in il:
            si = getattr(i, "sync_info", None)
            tn = type(i).__name__
            splittable = tn.startswith("Inst") and tn not in ("InstNoOp", "InstAllEngineBarrier")
            if (splittable and si is not None
                    and si.on_wait is not None and len(si.on_wait) > 1):
                waits = list(si.on_wait)
                for wi, w in enumerate(waits[:-1]):
                    out.append(mybir.InstNoOp(
                        name=f"{i.name}-wsplit{wi}", engine=i.engine,
                        sync_info=mybir.SyncInfo(on_wait=[w], on_update=[])))
                i.sync_info = mybir.SyncInfo(
                    on_wait=[waits[-1]], on_update=list(si.on_update))
                changed = True
            out.append(i)
        if changed:
            bb.instructions = out


def _prep_inputs(x, sqi, w_qkv, w_out, b_out, w_conv, b_conv, ln_gamma, ln_beta):
    x = np.asarray(x, np.float32)
    sqi = np.asarray(sqi, np.float32)
    w_qkv = np.asarray(w_qkv, np.float32)
    w_out = np.asarray(w_out, np.float32)
    b_out = np.asarray(b_out, np.float32)
    w_conv = np.asarray(w_conv, np.float32)
    b_conv = np.asarray(b_conv, np.float32)
    ln_gamma = np.asarray(ln_gamma, np.float32)
    ln_beta = np.asarray(ln_beta, np.float32)

    sp = np.pad(sqi, ((0, 0), (1, 1)))
    bias = (w_conv[0] * sp[:, :-2] + w_conv[1] * sp[:, 1:-1]
            + w_conv[2] * sp[:, 2:] + b_conv)                    # (B, T)

    wqT = (w_qkv[:C].T * SCALE).astype(bf16)
    wkT = w_qkv[C:2 * C].T.astype(bf16)
    wvT = w_qkv[2 * C:].T.astype(bf16)
    woT = w_out.T.astype(bf16)
    gm = ln_gamma.reshape(4, 128).T.copy().astype(np.float32)
    bt = ln_beta.reshape(4, 128).T.copy().astype(np.float32)
    iden = np.eye(128, dtype=bf16)

    in_maps = []
    for core in range(8):
        b, qh = divmod(core, 2)
        qs = slice(qh * TQ, (qh + 1) * TQ)
        cb = bias[b].reshape(KC, 128).T.copy().astype(np.float32)
        in_maps.append({
            "xct": x[b].astype(bf16),
            "xq": x[b][:, qs].copy().astype(bf16),
            "xseq": (x[b].T[qs] + b_out).copy().astype(np.float32),
            "wq": wqT, "wk": wkT, "wv": wvT, "wo": woT,
            "cb": cb, "gmm": gm, "bet": bt, "iden": iden,
        })
    return in_maps


def kernel(x, sqi, w_qkv, w_out, b_out, w_conv, b_conv, ln_gamma, ln_beta,
           _trace=False):
    if "nc" not in _CACHE:
        _CACHE["nc"] = _build_nc()
    nc = _CACHE["nc"]
    in_maps = _prep_inputs(x, sqi, w_qkv, w_out, b_out, w_conv, b_conv,
                           ln_gamma, ln_beta)
    res = run_bass_kernel_spmd(nc, in_maps, core_ids=list(range(8)), trace=_trace)
    _CACHE["last_result"] = res
    out = np.empty((B, C, T), np.float32)
    for core in range(8):
        b, qh = divmod(core, 2)
        out[b][:, qh * TQ:(qh + 1) * TQ] = res.results[core]["out"]
    return out
